# revision 2
# baseline (speedup 1.0000x reference)
"""Trainium2 Bass kernel for nn_Attention_3GIN2 (GIN aggregation + per-head attention).

Reference computation (b=4, t=1024, dim=256, 8 heads of d=32):
    xh  = x reshaped to [b, h, t, d]
    agg = (1+eps)*xh + adj @ xh                    (GIN aggregation, per head)
    qkv = agg @ W_qkv ; q,k,v = split(qkv)
    attn = softmax(q*dim^-0.5 @ k.T)               (per head, returned as output!)
    out  = gelu((attn @ v) reshaped to [b, t, dim])

Sharding: 8 cores = 4 batches x 2 head-groups (4 heads each). Each core computes
its (b, 4-head) slice entirely on-chip and writes its 16MB attn chunk + out slab.

Device-side layout strategy (everything f32):
  - adj.T (host-transposed) streams in; aggT[hd,t] = xh4.T @ adj.T + (1+eps)xT
    computed directly in "transposed" orientation so qk projections are natural.
  - qT/kT[32,t] per head from small matmuls against replicated W (q pre-scaled).
  - scores computed in BOTH orientations ([t,s] for the softmax/attn output,
    [s,t] for the attn@v contraction); softmax skips max-subtraction (scores
    bounded ~|25|, exp safely in f32 range) so exp(scores) needs no extra pass,
    with ACT accum_out providing row sums for free.
  - o computed transposed (oT = v.T @ exp(scoresT)), normalized after the
    PE transpose back to [t,(h d)] using per-partition 1/rowsum, gelu on ACT.
"""

import numpy as np

HEADS = 8
B = 4
T = 1024
DIM = 256
D = 32  # head dim
NH = 4  # heads per core
P = 128
NT = T // P  # 8 row tiles
SC = 512  # matmul free-dim chunk
SCALE = float(DIM) ** -0.5

_CACHE = {}


def _build():
    """Trace the per-core Bass program (identical on all 8 cores)."""
    import concourse.bass as bass
    import concourse.mybir as mybir
    import concourse.tile as tile
    from concourse import bacc
    from concourse.masks import make_identity

    f32 = mybir.dt.float32
    EXP = mybir.ActivationFunctionType.Exp
    GELU = mybir.ActivationFunctionType.Gelu

    nc = bacc.Bacc("TRN2", target_bir_lowering=False, debug=False)

    adjT_d = nc.dram_tensor("adjT", (T, T), f32, kind="ExternalInput").ap()
    xs_d = nc.dram_tensor("xs", (T, P), f32, kind="ExternalInput").ap()
    xsT_d = nc.dram_tensor("xsT", (P, T), f32, kind="ExternalInput").ap()
    w4_d = nc.dram_tensor("w4", (P, 2 * D), f32, kind="ExternalInput").ap()
    wblk_d = nc.dram_tensor("wblk", (P, P), f32, kind="ExternalInput").ap()
    eps1_d = nc.dram_tensor("eps1", (P, 1), f32, kind="ExternalInput").ap()
    attn_d = nc.dram_tensor("attn_o", (NH, T, T), f32, kind="ExternalOutput").ap()
    out_d = nc.dram_tensor("out_o", (T, P), f32, kind="ExternalOutput").ap()

    with tile.TileContext(nc) as tc:
        with (
            tc.tile_pool(name="const", bufs=1) as constp,
            tc.tile_pool(name="mainp", bufs=1) as mainp,
        ):
            # ---- constants / small inputs
            ident = constp.tile([P, P], f32)
            make_identity(nc, ident)
            w4 = constp.tile([P, 2 * D], f32)
            nc.sync.dma_start(w4, w4_d)
            wblk = constp.tile([P, P], f32)
            nc.sync.dma_start(wblk, wblk_d)
            eps1 = constp.tile([P, 1], f32)
            nc.sync.dma_start(eps1, eps1_d)

            # ---- x (natural, for GIN lhsT) and (1+eps) * x.T
            xh4 = mainp.tile([P, NT, P], f32)  # [s_p, sn, (h d)]
            nc.sync.dma_start(xh4, xs_d.rearrange("(n p) c -> p n c", p=P))
            xsTs = mainp.tile([P, T], f32)  # [(h d), t]
            nc.sync.dma_start(xsTs, xsT_d)
            sxT = mainp.tile([P, T], f32)
            nc.vector.tensor_scalar_mul(sxT, xsTs, eps1[:, 0:1])

            aggT = mainp.tile([P, T], f32)  # [(h d), t]
            qT = mainp.tile([D, NH * T], f32)  # q.T * scale, [d, (h t)]
            kT = mainp.tile([D, NH * T], f32)
            v3 = mainp.tile([P, NT, P], f32)  # v natural [t_p, tn, (h d)]
            denom = mainp.tile([P, NH * NT], f32)  # softmax row sums [t_p, (h tn)]
            recip = mainp.tile([P, NH * NT], f32)
            oT_sb = mainp.tile([P, T], f32)  # [(h d), t] unnormalized o.T
            ofin = mainp.tile([P, NT, P], f32)  # gelu(out) [t_p, tn, (h d)]

            # ---- GIN aggregation: aggT = xh4.T @ adjT + (1+eps)*x.T
            with (
                tc.tile_pool(name="adjp", bufs=1) as adjp,
                tc.tile_pool(name="spsum", bufs=2, space="PSUM") as spsum,
            ):
                adjT3 = adjp.tile([P, NT, T], f32)  # [s_p, sn, t]
                for c in range(4):
                    nc.sync.dma_start(
                        adjT3[:, 2 * c : 2 * c + 2, :],
                        adjT_d[2 * c * P : (2 * c + 2) * P, :].rearrange(
                            "(n p) t -> p n t", p=P
                        ),
                    )
                for tch in range(2):
                    agg_ps = spsum.tile([P, SC], f32, tag="agg")
                    for sn in range(NT):
                        nc.tensor.matmul(
                            agg_ps,
                            lhsT=xh4[:, sn, :],
                            rhs=adjT3[:, sn, tch * SC : (tch + 1) * SC],
                            start=(sn == 0),
                            stop=(sn == NT - 1),
                        )
                    nc.vector.tensor_add(
                        aggT[:, tch * SC : (tch + 1) * SC],
                        agg_ps,
                        sxT[:, tch * SC : (tch + 1) * SC],
                    )

                # ---- q/k projections: qkT = w4.T @ aggT (per head, K=32)
                for h in range(NH):
                    for tch in range(2):
                        qk_ps = spsum.tile([2 * D, SC], f32, tag="qk")
                        nc.tensor.matmul(
                            qk_ps,
                            lhsT=w4[h * D : (h + 1) * D, :],
                            rhs=aggT[h * D : (h + 1) * D, tch * SC : (tch + 1) * SC],
                            start=True,
                            stop=True,
                            tile_position=(h * D, 0),
                        )
                        nc.vector.tensor_copy(
                            qT[:, h * T + tch * SC : h * T + (tch + 1) * SC],
                            qk_ps[0:D, :],
                        )
                        nc.vector.tensor_copy(
                            kT[:, h * T + tch * SC : h * T + (tch + 1) * SC],
                            qk_ps[D : 2 * D, :],
                        )

                # ---- v (natural layout) via block-diagonal W_v
                for tn in range(NT):
                    v_ps = spsum.tile([P, P], f32, tag="v")
                    nc.tensor.matmul(
                        v_ps,
                        lhsT=aggT[:, tn * P : (tn + 1) * P],
                        rhs=wblk,
                        start=True,
                        stop=True,
                    )
                    nc.vector.tensor_copy(v3[:, tn, :], v_ps)

            # ---- main attention loops
            with (
                tc.tile_pool(name="scorep", bufs=2, space="PSUM") as scorep,
                tc.tile_pool(name="scoreTp", bufs=1, space="PSUM") as scoreTp,
                tc.tile_pool(name="oTp", bufs=1, space="PSUM") as oTp,
                tc.tile_pool(name="onp", bufs=1, space="PSUM") as onp,
                tc.tile_pool(name="epool", bufs=4) as epool,
                tc.tile_pool(name="attnp", bufs=2) as attnp,
                tc.tile_pool(name="etp", bufs=1) as etp,
                tc.tile_pool(name="onrmp", bufs=2) as onrmp,
            ):
                ET = etp.tile([P, NT, T], f32)  # exp(scores.T) [s_p, sn, t]
                for h in range(NH):
                    # attention-weights path: scores[t,s] -> exp -> normalize -> DMA
                    for tn in range(NT):
                        sc_ps = scorep.tile([P, T], f32, tag="sc")
                        for sch in range(2):
                            nc.tensor.matmul(
                                sc_ps[:, sch * SC : (sch + 1) * SC],
                                lhsT=qT[:, h * T + tn * P : h * T + (tn + 1) * P],
                                rhs=kT[:, h * T + sch * SC : h * T + (sch + 1) * SC],
                                start=True,
                                stop=True,
                            )
                        E = epool.tile([P, T], f32, tag="E")
                        idx = h * NT + tn
                        nc.scalar.activation(
                            E, sc_ps, EXP, accum_out=denom[:, idx : idx + 1]
                        )
                        nc.vector.reciprocal(
                            recip[:, idx : idx + 1], denom[:, idx : idx + 1]
                        )
                        if tn % 4 == 0:
                            a4 = attnp.tile([P, 4, T], f32, tag="a4")
                        nc.vector.tensor_scalar_mul(
                            a4[:, tn % 4, :], E, recip[:, idx : idx + 1]
                        )
                        if tn % 4 == 3:
                            q4 = tn // 4
                            nc.sync.dma_start(
                                attn_d[h, q4 * 4 * P : (q4 + 1) * 4 * P, :].rearrange(
                                    "(n p) s -> p n s", p=P
                                ),
                                a4,
                            )
                    # o path: scores.T -> exp -> oT = v.T @ exp(scores.T)
                    for sn in range(NT):
                        scT_ps = scoreTp.tile([P, T], f32, tag="scT")
                        for tch in range(2):
                            nc.tensor.matmul(
                                scT_ps[:, tch * SC : (tch + 1) * SC],
                                lhsT=kT[:, h * T + sn * P : h * T + (sn + 1) * P],
                                rhs=qT[:, h * T + tch * SC : h * T + (tch + 1) * SC],
                                start=True,
                                stop=True,
                            )
                        nc.scalar.activation(ET[:, sn, :], scT_ps, EXP)
                    for tch in range(2):
                        oT_ps = oTp.tile([D, SC], f32, tag="oT")
                        for sn in range(NT):
                            nc.tensor.matmul(
                                oT_ps,
                                lhsT=v3[:, sn, h * D : (h + 1) * D],
                                rhs=ET[:, sn, tch * SC : (tch + 1) * SC],
                                start=(sn == 0),
                                stop=(sn == NT - 1),
                            )
                        nc.vector.tensor_copy(
                            oT_sb[h * D : (h + 1) * D, tch * SC : (tch + 1) * SC],
                            oT_ps,
                        )

                # ---- out = gelu(o * recip) back in natural layout
                for tn in range(NT):
                    on_ps = onp.tile([P, P], f32, tag="on")
                    nc.tensor.transpose(on_ps, oT_sb[:, tn * P : (tn + 1) * P], ident)
                    onrm = onrmp.tile([P, NH, D], f32, tag="onrm")
                    rec4 = recip[:, tn : NH * NT : NT]  # [P, NH] head-major
                    nc.vector.tensor_tensor(
                        onrm,
                        on_ps.rearrange("p (h d) -> p h d", h=NH),
                        rec4[:, :, None].to_broadcast([P, NH, D]),
                        mybir.AluOpType.mult,
                    )
                    nc.scalar.activation(
                        ofin[:, tn, :], onrm.rearrange("p h d -> p (h d)"), GELU
                    )
                nc.sync.dma_start(out_d.rearrange("(n p) c -> p n c", p=P), ofin)

    nc.compile()
    return nc


def _prep_inputs(x, adj, W_qkv, eps):
    """Host-side shard/layout prep: one input map per core."""
    eps1 = np.full((P, 1), 1.0 + float(np.asarray(eps).reshape(-1)[0]), np.float32)
    wq = np.ascontiguousarray(W_qkv[:, 0:D]) * np.float32(SCALE)
    wk = np.ascontiguousarray(W_qkv[:, D : 2 * D])
    wv = np.ascontiguousarray(W_qkv[:, 2 * D : 3 * D])
    w4 = np.zeros((P, 2 * D), np.float32)
    wblk = np.zeros((P, P), np.float32)
    for h in range(NH):
        w4[h * D : (h + 1) * D, 0:D] = wq
        w4[h * D : (h + 1) * D, D : 2 * D] = wk
        wblk[h * D : (h + 1) * D, h * D : (h + 1) * D] = wv

    in_maps = []
    for core in range(8):
        b, hg = core // 2, core % 2
        xs = np.ascontiguousarray(x[b, :, hg * P : (hg + 1) * P])
        in_maps.append(
            {
                "adjT": np.ascontiguousarray(adj[b].T),
                "xs": xs,
                "xsT": np.ascontiguousarray(xs.T),
                "w4": w4,
                "wblk": wblk,
                "eps1": eps1,
            }
        )
    return in_maps


def run(x, adj, W_qkv, eps, trace=False):
    """Run on 8 NeuronCores; returns (out, attn_weight, BassKernelResults)."""
    from concourse import bass_utils

    if "nc" not in _CACHE:
        _CACHE["nc"] = _build()
    nc = _CACHE["nc"]

    in_maps = _prep_inputs(
        np.asarray(x, np.float32), np.asarray(adj, np.float32),
        np.asarray(W_qkv, np.float32), np.asarray(eps, np.float32),
    )
    res = bass_utils.run_bass_kernel_spmd(
        nc, in_maps, core_ids=list(range(8)), trace=trace
    )

    out = np.empty((B, T, DIM), np.float32)
    attn = np.empty((B, HEADS, T, T), np.float32)
    for core in range(8):
        b, hg = core // 2, core % 2
        r = res.results[core]
        attn[b, hg * NH : (hg + 1) * NH] = r["attn_o"]
        out[b, :, hg * P : (hg + 1) * P] = r["out_o"]
    return out, attn, res


def kernel(x, adj, rep_adj_dis, W_qkv, eps):
    out, attn, _ = run(x, adj, W_qkv, eps, trace=False)
    return out, attn


# revision 7
# speedup vs baseline: 1.3016x; 1.3016x over previous
"""Trainium2 Bass kernel for nn_Attention_3GIN2 (GIN aggregation + per-head attention).

Reference computation (b=4, t=1024, dim=256, 8 heads of d=32):
    xh  = x reshaped to [b, h, t, d]
    agg = (1+eps)*xh + adj @ xh                    (GIN aggregation, per head)
    qkv = agg @ W_qkv ; q,k,v = split(qkv)
    attn = softmax(q*dim^-0.5 @ k.T)               (per head, returned as output!)
    out  = gelu((attn @ v) reshaped to [b, t, dim])

Sharding: 8 cores = 4 batches x 2 head-groups (4 heads each). Each core computes
its (b, 4-head) slice entirely on-chip and writes its 16MB attn chunk + out slab.

Device-side layout strategy (everything f32):
  - adj.T (host-transposed) streams in; aggT[hd,t] = xh4.T @ adj.T + (1+eps)xT
    computed directly in "transposed" orientation so qk projections are natural.
  - qT/kT[32,t] per head from small matmuls against replicated W (q pre-scaled).
  - scores computed in BOTH orientations ([t,s] for the softmax/attn output,
    [s,t] for the attn@v contraction); softmax skips max-subtraction (scores
    bounded ~|25|, exp safely in f32 range) so exp(scores) needs no extra pass,
    with ACT accum_out providing row sums for free.
  - o computed transposed (oT = v.T @ exp(scoresT)), normalized after the
    PE transpose back to [t,(h d)] using per-partition 1/rowsum, gelu on ACT.
"""

import numpy as np

HEADS = 8
B = 4
T = 1024
DIM = 256
D = 32  # head dim
NH = 4  # heads per core
P = 128
NT = T // P  # 8 row tiles
SC = 512  # matmul free-dim chunk
SCALE = float(DIM) ** -0.5

_CACHE = {}


def _build():
    """Trace the per-core Bass program (identical on all 8 cores)."""
    import concourse.bass as bass
    import concourse.mybir as mybir
    import concourse.tile as tile
    from concourse import bacc
    from concourse.masks import make_identity

    f32 = mybir.dt.float32
    f32r = mybir.dt.float32r
    EXP = mybir.ActivationFunctionType.Exp
    GELU = mybir.ActivationFunctionType.Gelu

    nc = bacc.Bacc("TRN2", target_bir_lowering=False, debug=False)

    adjT_d = nc.dram_tensor("adjT", (T, T), f32, kind="ExternalInput").ap()
    xs_d = nc.dram_tensor("xs", (T, P), f32, kind="ExternalInput").ap()
    xsT_d = nc.dram_tensor("xsT", (P, T), f32, kind="ExternalInput").ap()
    w4_d = nc.dram_tensor("w4", (P, 2 * D), f32, kind="ExternalInput").ap()
    wblk_d = nc.dram_tensor("wblk", (P, P), f32, kind="ExternalInput").ap()
    eps1_d = nc.dram_tensor("eps1", (P, 1), f32, kind="ExternalInput").ap()
    attn_d = nc.dram_tensor("attn_o", (NH, T, T), f32, kind="ExternalOutput").ap()
    out_d = nc.dram_tensor("out_o", (T, P), f32, kind="ExternalOutput").ap()

    with tile.TileContext(nc) as tc:
        with (
            tc.tile_pool(name="const", bufs=1) as constp,
            tc.tile_pool(name="mainp", bufs=1) as mainp,
        ):
            # ---- constants / small inputs
            ident = constp.tile([P, P], f32)
            make_identity(nc, ident)
            w4 = constp.tile([P, 2 * D], f32)
            nc.sync.dma_start(w4, w4_d)
            wblk = constp.tile([P, P], f32)
            nc.sync.dma_start(wblk, wblk_d)
            eps1 = constp.tile([P, 1], f32)
            nc.sync.dma_start(eps1, eps1_d)

            # ---- x (natural, for GIN lhsT) and (1+eps) * x.T
            xh4 = mainp.tile([P, NT, P], f32)  # [s_p, sn, (h d)]
            nc.sync.dma_start(xh4, xs_d.rearrange("(n p) c -> p n c", p=P))
            xsTs = mainp.tile([P, T], f32)  # [(h d), t]
            nc.sync.dma_start(xsTs, xsT_d)
            sxT = mainp.tile([P, T], f32)
            nc.vector.tensor_scalar_mul(sxT, xsTs, eps1[:, 0:1])

            aggT = mainp.tile([P, T], f32)  # [(h d), t]
            # q.T/k.T in 4 head-strips (partitions 32h..32h+32 = head h) so
            # consecutive score matmuls hit different PE row-groups and their
            # LDWEIGHTS overlap in-flight MATMULs.
            qT = mainp.tile([P, T], f32)  # [(h d), t], q pre-scaled
            kT = mainp.tile([P, T], f32)
            v3 = mainp.tile([P, NT, P], f32r)  # v natural [t_p, tn, (h d)]
            denom = mainp.tile([P, NH * NT], f32)  # softmax row sums [t_p, (h tn)]
            recip = mainp.tile([P, NH * NT], f32)
            oT_sb = mainp.tile([P, T], f32)  # [(h d), t] unnormalized o.T
            ofin = mainp.tile([P, NT, P], f32)  # gelu(out) [t_p, tn, (h d)]

            # ---- GIN aggregation: aggT = xh4.T @ adjT + (1+eps)*x.T
            with (
                tc.tile_pool(name="adjp", bufs=1) as adjp,
                tc.tile_pool(name="spsum", bufs=2, space="PSUM") as spsum,
            ):
                adjT3 = adjp.tile([P, NT, T], f32)  # [s_p, sn, t]
                for c in range(4):
                    nc.sync.dma_start(
                        adjT3[:, 2 * c : 2 * c + 2, :],
                        adjT_d[2 * c * P : (2 * c + 2) * P, :].rearrange(
                            "(n p) t -> p n t", p=P
                        ),
                    )
                for tch in range(2):
                    agg_ps = spsum.tile([P, SC], f32, tag="agg")
                    for sn in range(NT):
                        nc.tensor.matmul(
                            agg_ps,
                            lhsT=xh4[:, sn, :],
                            rhs=adjT3[:, sn, tch * SC : (tch + 1) * SC],
                            start=(sn == 0),
                            stop=(sn == NT - 1),
                        )
                    nc.vector.tensor_add(
                        aggT[:, tch * SC : (tch + 1) * SC],
                        agg_ps,
                        sxT[:, tch * SC : (tch + 1) * SC],
                    )

                # ---- q/k projections: qkT = w4.T @ aggT (per head, K=32)
                for h in range(NH):
                    for tch in range(2):
                        qk_ps = spsum.tile([2 * D, SC], f32, tag="qk")
                        nc.tensor.matmul(
                            qk_ps,
                            lhsT=w4[h * D : (h + 1) * D, :],
                            rhs=aggT[h * D : (h + 1) * D, tch * SC : (tch + 1) * SC],
                            start=True,
                            stop=True,
                            tile_position=(h * D, 0),
                        )
                        nc.vector.tensor_copy(
                            qT[h * D : (h + 1) * D, tch * SC : (tch + 1) * SC],
                            qk_ps[0:D, :],
                        )
                        nc.vector.tensor_copy(
                            kT[h * D : (h + 1) * D, tch * SC : (tch + 1) * SC],
                            qk_ps[D : 2 * D, :],
                        )

                # ---- v (natural layout) via block-diagonal W_v
                for tn in range(NT):
                    v_ps = spsum.tile([P, P], f32, tag="v")
                    nc.tensor.matmul(
                        v_ps,
                        lhsT=aggT[:, tn * P : (tn + 1) * P],
                        rhs=wblk,
                        start=True,
                        stop=True,
                    )
                    nc.vector.tensor_copy(v3[:, tn, :], v_ps)

            # ---- main attention loops
            with (
                tc.tile_pool(name="mmps", bufs=4, space="PSUM") as mmps,
                tc.tile_pool(name="epool", bufs=4) as epool,
                tc.tile_pool(name="attnp", bufs=2) as attnp,
                tc.tile_pool(name="etp", bufs=1) as etp,
                tc.tile_pool(name="onrmp", bufs=2) as onrmp,
            ):
                ET = etp.tile([P, NT, T], f32r)  # exp(scores.T) [s_p, sn, t]
                for h in range(NH):
                    hs = slice(h * D, (h + 1) * D)  # this head's row strip
                    # attention-weights path: scores[t,s] -> exp -> normalize -> DMA
                    for tn in range(NT):
                        sc_ps = mmps.tile([P, T], f32, tag="sc", name=f"sc{h}_{tn}")
                        for sch in range(2):
                            nc.tensor.matmul(
                                sc_ps[:, sch * SC : (sch + 1) * SC],
                                lhsT=qT[hs, tn * P : (tn + 1) * P],
                                rhs=kT[hs, sch * SC : (sch + 1) * SC],
                                start=True,
                                stop=True,
                                tile_position=(h * D, 0),
                            )
                        E = epool.tile([P, T], f32, tag="E")
                        idx = h * NT + tn
                        nc.scalar.activation(
                            E, sc_ps, EXP, accum_out=denom[:, idx : idx + 1]
                        )
                        nc.vector.reciprocal(
                            recip[:, idx : idx + 1], denom[:, idx : idx + 1]
                        )
                        if tn % 4 == 0:
                            a4 = attnp.tile([P, 4, T], f32, tag="a4")
                        nc.vector.tensor_scalar_mul(
                            a4[:, tn % 4, :], E, recip[:, idx : idx + 1]
                        )
                        if tn % 4 == 3:
                            q4 = tn // 4
                            nc.sync.dma_start(
                                attn_d[h, q4 * 4 * P : (q4 + 1) * 4 * P, :].rearrange(
                                    "(n p) s -> p n s", p=P
                                ),
                                a4,
                            )
                    # o path: scores.T -> exp(fp32r) -> oT = v.T @ exp(scores.T)
                    for sn in range(NT):
                        scT_ps = mmps.tile([P, T], f32, tag="sc", name=f"scT{h}_{sn}")
                        for tch in range(2):
                            nc.tensor.matmul(
                                scT_ps[:, tch * SC : (tch + 1) * SC],
                                lhsT=kT[hs, sn * P : (sn + 1) * P],
                                rhs=qT[hs, tch * SC : (tch + 1) * SC],
                                start=True,
                                stop=True,
                                tile_position=(h * D, 0),
                            )
                        nc.scalar.activation(ET[:, sn, :], scT_ps, EXP)
                    for tch in range(2):
                        oT_ps = mmps.tile([D, SC], f32, tag="sc", name=f"oT{h}_{tch}")
                        for sn in range(NT):
                            nc.tensor.matmul(
                                oT_ps,
                                lhsT=v3[:, sn, hs],
                                rhs=ET[:, sn, tch * SC : (tch + 1) * SC],
                                start=(sn == 0),
                                stop=(sn == NT - 1),
                            )
                        nc.vector.tensor_copy(
                            oT_sb[hs, tch * SC : (tch + 1) * SC],
                            oT_ps,
                        )

                # ---- out = gelu(o * recip) back in natural layout
                for tn in range(NT):
                    on_ps = mmps.tile([P, P], f32, tag="sc", name=f"on{tn}")
                    nc.tensor.transpose(on_ps, oT_sb[:, tn * P : (tn + 1) * P], ident)
                    onrm = onrmp.tile([P, NH, D], f32, tag="onrm")
                    rec4 = recip[:, tn : NH * NT : NT]  # [P, NH] head-major
                    nc.vector.tensor_tensor(
                        onrm,
                        on_ps.rearrange("p (h d) -> p h d", h=NH),
                        rec4[:, :, None].to_broadcast([P, NH, D]),
                        mybir.AluOpType.mult,
                    )
                    nc.scalar.activation(
                        ofin[:, tn, :], onrm.rearrange("p h d -> p (h d)"), GELU
                    )
                nc.sync.dma_start(out_d.rearrange("(n p) c -> p n c", p=P), ofin)

    nc.compile()
    return nc


def _prep_inputs(x, adj, W_qkv, eps):
    """Host-side shard/layout prep: one input map per core."""
    eps1 = np.full((P, 1), 1.0 + float(np.asarray(eps).reshape(-1)[0]), np.float32)
    wq = np.ascontiguousarray(W_qkv[:, 0:D]) * np.float32(SCALE)
    wk = np.ascontiguousarray(W_qkv[:, D : 2 * D])
    wv = np.ascontiguousarray(W_qkv[:, 2 * D : 3 * D])
    w4 = np.zeros((P, 2 * D), np.float32)
    wblk = np.zeros((P, P), np.float32)
    for h in range(NH):
        w4[h * D : (h + 1) * D, 0:D] = wq
        w4[h * D : (h + 1) * D, D : 2 * D] = wk
        wblk[h * D : (h + 1) * D, h * D : (h + 1) * D] = wv

    in_maps = []
    for core in range(8):
        b, hg = core // 2, core % 2
        xs = np.ascontiguousarray(x[b, :, hg * P : (hg + 1) * P])
        in_maps.append(
            {
                "adjT": np.ascontiguousarray(adj[b].T),
                "xs": xs,
                "xsT": np.ascontiguousarray(xs.T),
                "w4": w4,
                "wblk": wblk,
                "eps1": eps1,
            }
        )
    return in_maps


def run(x, adj, W_qkv, eps, trace=False):
    """Run on 8 NeuronCores; returns (out, attn_weight, BassKernelResults)."""
    from concourse import bass_utils

    if "nc" not in _CACHE:
        _CACHE["nc"] = _build()
    nc = _CACHE["nc"]

    in_maps = _prep_inputs(
        np.asarray(x, np.float32), np.asarray(adj, np.float32),
        np.asarray(W_qkv, np.float32), np.asarray(eps, np.float32),
    )
    res = bass_utils.run_bass_kernel_spmd(
        nc, in_maps, core_ids=list(range(8)), trace=trace
    )

    out = np.empty((B, T, DIM), np.float32)
    attn = np.empty((B, HEADS, T, T), np.float32)
    for core in range(8):
        b, hg = core // 2, core % 2
        r = res.results[core]
        attn[b, hg * NH : (hg + 1) * NH] = r["attn_o"]
        out[b, :, hg * P : (hg + 1) * P] = r["out_o"]
    return out, attn, res


def kernel(x, adj, rep_adj_dis, W_qkv, eps):
    out, attn, _ = run(x, adj, W_qkv, eps, trace=False)
    return out, attn


# revision 9
# speedup vs baseline: 2.0435x; 1.5700x over previous
"""Trainium2 Bass kernel for nn_Attention_3GIN2 (GIN aggregation + per-head attention).

Reference computation (b=4, t=1024, dim=256, 8 heads of d=32):
    xh  = x reshaped to [b, h, t, d]
    agg = (1+eps)*xh + adj @ xh                    (GIN aggregation, per head)
    qkv = agg @ W_qkv ; q,k,v = split(qkv)
    attn = softmax(q*dim^-0.5 @ k.T)               (per head, returned as output!)
    out  = gelu((attn @ v) reshaped to [b, t, dim])

Sharding: 8 cores = 4 batches x 2 head-groups (4 heads each). Each core computes
its (b, 4-head) slice entirely on-chip and writes its 16MB attn chunk + out slab.

Device-side layout strategy (everything f32):
  - adj.T (host-transposed) streams in; aggT[hd,t] = xh4.T @ adj.T + (1+eps)xT
    computed directly in "transposed" orientation so qk projections are natural.
  - qT/kT[32,t] per head from small matmuls against replicated W (q pre-scaled).
  - scores computed in BOTH orientations ([t,s] for the softmax/attn output,
    [s,t] for the attn@v contraction); softmax skips max-subtraction (scores
    bounded ~|25|, exp safely in f32 range) so exp(scores) needs no extra pass,
    with ACT accum_out providing row sums for free.
  - o computed transposed (oT = v.T @ exp(scoresT)), normalized after the
    PE transpose back to [t,(h d)] using per-partition 1/rowsum, gelu on ACT.
"""

import numpy as np

HEADS = 8
B = 4
T = 1024
DIM = 256
D = 32  # head dim
NH = 4  # heads per core
P = 128
NT = T // P  # 8 row tiles
SC = 512  # matmul free-dim chunk
SCALE = float(DIM) ** -0.5

_CACHE = {}


def _build():
    """Trace the per-core Bass program (identical on all 8 cores)."""
    import concourse.bass as bass
    import concourse.mybir as mybir
    import concourse.tile as tile
    from concourse import bacc
    from concourse.masks import make_identity

    f32 = mybir.dt.float32
    f32r = mybir.dt.float32r
    EXP = mybir.ActivationFunctionType.Exp
    GELU = mybir.ActivationFunctionType.Gelu

    nc = bacc.Bacc("TRN2", target_bir_lowering=False, debug=False)

    adjT_d = nc.dram_tensor("adjT", (T, T), f32, kind="ExternalInput").ap()
    xs_d = nc.dram_tensor("xs", (T, P), f32, kind="ExternalInput").ap()
    xsT_d = nc.dram_tensor("xsT", (P, T), f32, kind="ExternalInput").ap()
    w4_d = nc.dram_tensor("w4", (P, 2 * D), f32, kind="ExternalInput").ap()
    wblk_d = nc.dram_tensor("wblk", (P, P), f32, kind="ExternalInput").ap()
    eps1_d = nc.dram_tensor("eps1", (P, 1), f32, kind="ExternalInput").ap()
    attn_d = nc.dram_tensor("attn_o", (NH, T, T), f32, kind="ExternalOutput").ap()
    out_d = nc.dram_tensor("out_o", (T, P), f32, kind="ExternalOutput").ap()

    with tile.TileContext(nc) as tc:
        with (
            tc.tile_pool(name="const", bufs=1) as constp,
            tc.tile_pool(name="mainp", bufs=1) as mainp,
        ):
            # ---- constants / small inputs
            ident = constp.tile([P, P], f32)
            make_identity(nc, ident)
            w4 = constp.tile([P, 2 * D], f32)
            nc.sync.dma_start(w4, w4_d)
            wblk = constp.tile([P, P], f32)
            nc.sync.dma_start(wblk, wblk_d)
            eps1 = constp.tile([P, 1], f32)
            nc.sync.dma_start(eps1, eps1_d)

            # ---- x (natural, for GIN lhsT) and (1+eps) * x.T
            xh4 = mainp.tile([P, NT, P], f32)  # [s_p, sn, (h d)]
            nc.sync.dma_start(xh4, xs_d.rearrange("(n p) c -> p n c", p=P))
            xsTs = mainp.tile([P, T], f32)  # [(h d), t]
            nc.sync.dma_start(xsTs, xsT_d)
            sxT = mainp.tile([P, T], f32)
            nc.vector.tensor_scalar_mul(sxT, xsTs, eps1[:, 0:1])

            aggT = mainp.tile([P, T], f32)  # [(h d), t]
            # q.T/k.T in 4 head-strips (partitions 32h..32h+32 = head h) so
            # consecutive score matmuls hit different PE row-groups and their
            # LDWEIGHTS overlap in-flight MATMULs.
            qT = mainp.tile([P, T], f32)  # [(h d), t], q pre-scaled
            kT = mainp.tile([P, T], f32)
            v3 = mainp.tile([P, NT, P], f32r)  # v natural [t_p, tn, (h d)]
            denom = mainp.tile([P, NH * NT], f32)  # softmax row sums [t_p, (h tn)]
            recip = mainp.tile([P, NH * NT], f32)
            oT_sb = mainp.tile([P, T], f32)  # [(h d), t] unnormalized o.T
            ofin = mainp.tile([P, NT, P], f32)  # gelu(out) [t_p, tn, (h d)]

            # ---- GIN aggregation: aggT = xh4.T @ adjT + (1+eps)*x.T
            with (
                tc.tile_pool(name="adjp", bufs=1) as adjp,
                tc.tile_pool(name="spsum", bufs=2, space="PSUM") as spsum,
            ):
                adjT3 = adjp.tile([P, NT, T], f32)  # [s_p, sn, t]
                for c in range(4):
                    nc.sync.dma_start(
                        adjT3[:, 2 * c : 2 * c + 2, :],
                        adjT_d[2 * c * P : (2 * c + 2) * P, :].rearrange(
                            "(n p) t -> p n t", p=P
                        ),
                    )
                for tch in range(2):
                    agg_ps = spsum.tile([P, SC], f32, tag="agg")
                    for sn in range(NT):
                        nc.tensor.matmul(
                            agg_ps,
                            lhsT=xh4[:, sn, :],
                            rhs=adjT3[:, sn, tch * SC : (tch + 1) * SC],
                            start=(sn == 0),
                            stop=(sn == NT - 1),
                        )
                    nc.vector.tensor_add(
                        aggT[:, tch * SC : (tch + 1) * SC],
                        agg_ps,
                        sxT[:, tch * SC : (tch + 1) * SC],
                    )

                # ---- q/k projections: qkT = w4.T @ aggT (per head, K=32)
                for h in range(NH):
                    for tch in range(2):
                        qk_ps = spsum.tile([2 * D, SC], f32, tag="qk")
                        nc.tensor.matmul(
                            qk_ps,
                            lhsT=w4[h * D : (h + 1) * D, :],
                            rhs=aggT[h * D : (h + 1) * D, tch * SC : (tch + 1) * SC],
                            start=True,
                            stop=True,
                            tile_position=(h * D, 0),
                        )
                        nc.vector.tensor_copy(
                            qT[h * D : (h + 1) * D, tch * SC : (tch + 1) * SC],
                            qk_ps[0:D, :],
                        )
                        nc.vector.tensor_copy(
                            kT[h * D : (h + 1) * D, tch * SC : (tch + 1) * SC],
                            qk_ps[D : 2 * D, :],
                        )

                # ---- v (natural layout) via block-diagonal W_v
                for tn in range(NT):
                    v_ps = spsum.tile([P, P], f32, tag="v")
                    nc.tensor.matmul(
                        v_ps,
                        lhsT=aggT[:, tn * P : (tn + 1) * P],
                        rhs=wblk,
                        start=True,
                        stop=True,
                    )
                    nc.vector.tensor_copy(v3[:, tn, :], v_ps)

            # fp32r copies of q/k for the o-path score matmuls (1-pass on PE)
            qTr = mainp.tile([P, T], f32r)
            nc.vector.tensor_copy(qTr, qT)
            kTr = mainp.tile([P, T], f32r)
            nc.vector.tensor_copy(kTr, kT)

            # ---- main attention loops (heads interleaved so consecutive
            # matmuls rotate PE row-groups and LDWEIGHTS overlaps)
            with (
                tc.tile_pool(name="mmps", bufs=3, space="PSUM") as mmps,
                tc.tile_pool(name="otps", bufs=2, space="PSUM") as otps,
                tc.tile_pool(name="epool", bufs=4) as epool,
                tc.tile_pool(name="attnp", bufs=6) as attnp,
                tc.tile_pool(name="etp", bufs=6) as etp,
                tc.tile_pool(name="onrmp", bufs=2) as onrmp,
            ):
                # attention-weights path: scores[t,s] -> exp -> normalize -> DMA
                a4s = {}
                for tn in range(NT):
                    for h in range(NH):
                        hs = slice(h * D, (h + 1) * D)
                        sc_ps = mmps.tile([P, T], f32, tag="sc", name=f"sc{h}_{tn}")
                        for sch in range(2):
                            nc.tensor.matmul(
                                sc_ps[:, sch * SC : (sch + 1) * SC],
                                lhsT=qT[hs, tn * P : (tn + 1) * P],
                                rhs=kT[hs, sch * SC : (sch + 1) * SC],
                                start=True,
                                stop=True,
                                tile_position=(h * D, 0),
                            )
                        E = epool.tile([P, T], f32, tag="E", name=f"E{h}_{tn}")
                        idx = h * NT + tn
                        nc.scalar.activation(
                            E, sc_ps, EXP, accum_out=denom[:, idx : idx + 1]
                        )
                        nc.vector.reciprocal(
                            recip[:, idx : idx + 1], denom[:, idx : idx + 1]
                        )
                        if tn % 4 == 0:
                            a4s[h] = attnp.tile(
                                [P, 4, T], f32, tag="a4", name=f"a4_{h}_{tn // 4}"
                            )
                        nc.vector.tensor_scalar_mul(
                            a4s[h][:, tn % 4, :], E, recip[:, idx : idx + 1]
                        )
                        if tn % 4 == 3:
                            q4 = tn // 4
                            nc.sync.dma_start(
                                attn_d[h, q4 * 4 * P : (q4 + 1) * 4 * P, :].rearrange(
                                    "(n p) s -> p n s", p=P
                                ),
                                a4s[h],
                            )
                # o path: scoresT (fp32r) -> exp(fp32r) -> oT = v.T @ exp(scoresT)
                ET = etp.tile([P, NT, T], f32r, bufs=1)  # exp(scores.T) [s_p, sn, t]
                for h in range(NH):
                    hs = slice(h * D, (h + 1) * D)
                    for sn in range(NT):
                        scT_ps = mmps.tile([P, T], f32, tag="sc", name=f"scT{h}_{sn}")
                        for tch in range(2):
                            nc.tensor.matmul(
                                scT_ps[:, tch * SC : (tch + 1) * SC],
                                lhsT=kTr[hs, sn * P : (sn + 1) * P],
                                rhs=qTr[hs, tch * SC : (tch + 1) * SC],
                                start=True,
                                stop=True,
                                tile_position=(h * D, 0),
                            )
                        nc.scalar.activation(ET[:, sn, :], scT_ps, EXP)
                    for tch in range(2):
                        oT_ps = otps.tile([D, SC], f32, tag="ot", name=f"ot{h}_{tch}")
                        for sn in range(NT):
                            nc.tensor.matmul(
                                oT_ps,
                                lhsT=v3[:, sn, hs],
                                rhs=ET[:, sn, tch * SC : (tch + 1) * SC],
                                start=(sn == 0),
                                stop=(sn == NT - 1),
                            )
                        nc.vector.tensor_copy(
                            oT_sb[hs, tch * SC : (tch + 1) * SC], oT_ps
                        )

                # ---- out = gelu(o * recip) back in natural layout
                for tn in range(NT):
                    on_ps = mmps.tile([P, P], f32, tag="sc", name=f"on{tn}")
                    nc.tensor.transpose(on_ps, oT_sb[:, tn * P : (tn + 1) * P], ident)
                    onrm = onrmp.tile([P, NH, D], f32, tag="onrm")
                    rec4 = recip[:, tn : NH * NT : NT]  # [P, NH] head-major
                    nc.vector.tensor_tensor(
                        onrm,
                        on_ps.rearrange("p (h d) -> p h d", h=NH),
                        rec4[:, :, None].to_broadcast([P, NH, D]),
                        mybir.AluOpType.mult,
                    )
                    nc.scalar.activation(
                        ofin[:, tn, :], onrm.rearrange("p h d -> p (h d)"), GELU
                    )
                nc.sync.dma_start(out_d.rearrange("(n p) c -> p n c", p=P), ofin)

    nc.compile()
    return nc


def _prep_inputs(x, adj, W_qkv, eps):
    """Host-side shard/layout prep: one input map per core."""
    eps1 = np.full((P, 1), 1.0 + float(np.asarray(eps).reshape(-1)[0]), np.float32)
    wq = np.ascontiguousarray(W_qkv[:, 0:D]) * np.float32(SCALE)
    wk = np.ascontiguousarray(W_qkv[:, D : 2 * D])
    wv = np.ascontiguousarray(W_qkv[:, 2 * D : 3 * D])
    w4 = np.zeros((P, 2 * D), np.float32)
    wblk = np.zeros((P, P), np.float32)
    for h in range(NH):
        w4[h * D : (h + 1) * D, 0:D] = wq
        w4[h * D : (h + 1) * D, D : 2 * D] = wk
        wblk[h * D : (h + 1) * D, h * D : (h + 1) * D] = wv

    in_maps = []
    for core in range(8):
        b, hg = core // 2, core % 2
        xs = np.ascontiguousarray(x[b, :, hg * P : (hg + 1) * P])
        in_maps.append(
            {
                "adjT": np.ascontiguousarray(adj[b].T),
                "xs": xs,
                "xsT": np.ascontiguousarray(xs.T),
                "w4": w4,
                "wblk": wblk,
                "eps1": eps1,
            }
        )
    return in_maps


def run(x, adj, W_qkv, eps, trace=False):
    """Run on 8 NeuronCores; returns (out, attn_weight, BassKernelResults)."""
    from concourse import bass_utils

    if "nc" not in _CACHE:
        _CACHE["nc"] = _build()
    nc = _CACHE["nc"]

    in_maps = _prep_inputs(
        np.asarray(x, np.float32), np.asarray(adj, np.float32),
        np.asarray(W_qkv, np.float32), np.asarray(eps, np.float32),
    )
    res = bass_utils.run_bass_kernel_spmd(
        nc, in_maps, core_ids=list(range(8)), trace=trace
    )

    out = np.empty((B, T, DIM), np.float32)
    attn = np.empty((B, HEADS, T, T), np.float32)
    for core in range(8):
        b, hg = core // 2, core % 2
        r = res.results[core]
        attn[b, hg * NH : (hg + 1) * NH] = r["attn_o"]
        out[b, :, hg * P : (hg + 1) * P] = r["out_o"]
    return out, attn, res


def kernel(x, adj, rep_adj_dis, W_qkv, eps):
    out, attn, _ = run(x, adj, W_qkv, eps, trace=False)
    return out, attn


# revision 11
# speedup vs baseline: 2.2197x; 1.0862x over previous
"""Trainium2 Bass kernel for nn_Attention_3GIN2 (GIN aggregation + per-head attention).

Reference computation (b=4, t=1024, dim=256, 8 heads of d=32):
    xh  = x reshaped to [b, h, t, d]
    agg = (1+eps)*xh + adj @ xh                    (GIN aggregation, per head)
    qkv = agg @ W_qkv ; q,k,v = split(qkv)
    attn = softmax(q*dim^-0.5 @ k.T)               (per head, returned as output!)
    out  = gelu((attn @ v) reshaped to [b, t, dim])

Sharding: 8 cores = 4 batches x 2 head-groups (4 heads each). Each core computes
its (b, 4-head) slice entirely on-chip and writes its 16MB attn chunk + out slab.

Device-side layout strategy (everything f32):
  - adj.T (host-transposed) streams in; aggT[hd,t] = xh4.T @ adj.T + (1+eps)xT
    computed directly in "transposed" orientation so qk projections are natural.
  - qT/kT[32,t] per head from small matmuls against replicated W (q pre-scaled).
  - scores computed in BOTH orientations ([t,s] for the softmax/attn output,
    [s,t] for the attn@v contraction); softmax skips max-subtraction (scores
    bounded ~|25|, exp safely in f32 range) so exp(scores) needs no extra pass,
    with ACT accum_out providing row sums for free.
  - o computed transposed (oT = v.T @ exp(scoresT)), normalized after the
    PE transpose back to [t,(h d)] using per-partition 1/rowsum, gelu on ACT.
"""

import numpy as np

HEADS = 8
B = 4
T = 1024
DIM = 256
D = 32  # head dim
NH = 4  # heads per core
P = 128
NT = T // P  # 8 row tiles
SC = 512  # matmul free-dim chunk
SCALE = float(DIM) ** -0.5

_CACHE = {}

# scores matmuls in float32r (PE 1-pass, ~2.7x faster than fp32's 4-pass;
# rounds q/k to ~15 mantissa bits -> attn error ~1e-3-scale). False = full fp32.
SCORES_F32R = True


def _build():
    """Trace the per-core Bass program (identical on all 8 cores)."""
    import concourse.bass as bass
    import concourse.mybir as mybir
    import concourse.tile as tile
    from concourse import bacc
    from concourse.masks import make_identity

    f32 = mybir.dt.float32
    f32r = mybir.dt.float32r
    EXP = mybir.ActivationFunctionType.Exp
    GELU = mybir.ActivationFunctionType.Gelu

    nc = bacc.Bacc("TRN2", target_bir_lowering=False, debug=False)

    adjT_d = nc.dram_tensor("adjT", (T, T), f32, kind="ExternalInput").ap()
    xs_d = nc.dram_tensor("xs", (T, P), f32, kind="ExternalInput").ap()
    xsT_d = nc.dram_tensor("xsT", (P, T), f32, kind="ExternalInput").ap()
    w4_d = nc.dram_tensor("w4", (P, 2 * D), f32, kind="ExternalInput").ap()
    wblk_d = nc.dram_tensor("wblk", (P, P), f32, kind="ExternalInput").ap()
    eps1_d = nc.dram_tensor("eps1", (P, 1), f32, kind="ExternalInput").ap()
    attn_d = nc.dram_tensor("attn_o", (NH, T, T), f32, kind="ExternalOutput").ap()
    out_d = nc.dram_tensor("out_o", (T, P), f32, kind="ExternalOutput").ap()

    with tile.TileContext(nc) as tc:
        with (
            tc.tile_pool(name="const", bufs=1) as constp,
            tc.tile_pool(name="mainp", bufs=1) as mainp,
        ):
            # ---- constants / small inputs
            ident = constp.tile([P, P], f32)
            make_identity(nc, ident)
            w4 = constp.tile([P, 2 * D], f32)
            nc.sync.dma_start(w4, w4_d)
            wblk = constp.tile([P, P], f32)
            nc.sync.dma_start(wblk, wblk_d)
            eps1 = constp.tile([P, 1], f32)
            nc.sync.dma_start(eps1, eps1_d)

            # ---- x (natural, for GIN lhsT) and (1+eps) * x.T
            xh4 = mainp.tile([P, NT, P], f32)  # [s_p, sn, (h d)]
            nc.sync.dma_start(xh4, xs_d.rearrange("(n p) c -> p n c", p=P))
            xsTs = mainp.tile([P, T], f32)  # [(h d), t]
            nc.sync.dma_start(xsTs, xsT_d)
            sxT = mainp.tile([P, T], f32)
            nc.vector.tensor_scalar_mul(sxT, xsTs, eps1[:, 0:1])

            aggT = mainp.tile([P, T], f32)  # [(h d), t]
            # q.T/k.T in 4 head-strips (partitions 32h..32h+32 = head h) so
            # consecutive score matmuls hit different PE row-groups and their
            # LDWEIGHTS overlap in-flight MATMULs.
            qT = mainp.tile([P, T], f32)  # [(h d), t], q pre-scaled
            kT = mainp.tile([P, T], f32)
            v3 = mainp.tile([P, NT, P], f32r)  # v natural [t_p, tn, (h d)]
            denom = mainp.tile([P, NH * NT], f32)  # softmax row sums [t_p, (h tn)]
            recip = mainp.tile([P, NH * NT], f32)
            oT_sb = mainp.tile([P, T], f32)  # [(h d), t] unnormalized o.T
            ofin = mainp.tile([P, NT, P], f32)  # gelu(out) [t_p, tn, (h d)]

            # ---- GIN aggregation: aggT = xh4.T @ adjT + (1+eps)*x.T
            with (
                tc.tile_pool(name="adjp", bufs=1) as adjp,
                tc.tile_pool(name="spsum", bufs=2, space="PSUM") as spsum,
            ):
                adjT3 = adjp.tile([P, NT, T], f32)  # [s_p, sn, t]
                for c in range(4):
                    nc.sync.dma_start(
                        adjT3[:, 2 * c : 2 * c + 2, :],
                        adjT_d[2 * c * P : (2 * c + 2) * P, :].rearrange(
                            "(n p) t -> p n t", p=P
                        ),
                    )
                for tch in range(2):
                    agg_ps = spsum.tile([P, SC], f32, tag="agg")
                    for sn in range(NT):
                        nc.tensor.matmul(
                            agg_ps,
                            lhsT=xh4[:, sn, :],
                            rhs=adjT3[:, sn, tch * SC : (tch + 1) * SC],
                            start=(sn == 0),
                            stop=(sn == NT - 1),
                        )
                    nc.vector.tensor_add(
                        aggT[:, tch * SC : (tch + 1) * SC],
                        agg_ps,
                        sxT[:, tch * SC : (tch + 1) * SC],
                    )

                # ---- q/k projections: qkT = w4.T @ aggT (per head, K=32)
                for h in range(NH):
                    for tch in range(2):
                        qk_ps = spsum.tile([2 * D, SC], f32, tag="qk")
                        nc.tensor.matmul(
                            qk_ps,
                            lhsT=w4[h * D : (h + 1) * D, :],
                            rhs=aggT[h * D : (h + 1) * D, tch * SC : (tch + 1) * SC],
                            start=True,
                            stop=True,
                            tile_position=(h * D, 0),
                        )
                        nc.vector.tensor_copy(
                            qT[h * D : (h + 1) * D, tch * SC : (tch + 1) * SC],
                            qk_ps[0:D, :],
                        )
                        nc.vector.tensor_copy(
                            kT[h * D : (h + 1) * D, tch * SC : (tch + 1) * SC],
                            qk_ps[D : 2 * D, :],
                        )

                # ---- v (natural layout) via block-diagonal W_v
                for tn in range(NT):
                    v_ps = spsum.tile([P, P], f32, tag="v")
                    nc.tensor.matmul(
                        v_ps,
                        lhsT=aggT[:, tn * P : (tn + 1) * P],
                        rhs=wblk,
                        start=True,
                        stop=True,
                    )
                    nc.vector.tensor_copy(v3[:, tn, :], v_ps)

            # fp32r copies of q/k for the o-path score matmuls (1-pass on PE)
            qTr = mainp.tile([P, T], f32r)
            nc.vector.tensor_copy(qTr, qT)
            kTr = mainp.tile([P, T], f32r)
            nc.vector.tensor_copy(kTr, kT)

            # ---- main attention loops (heads interleaved so consecutive
            # matmuls rotate PE row-groups and LDWEIGHTS overlaps)
            with (
                tc.tile_pool(name="mmps", bufs=3, space="PSUM") as mmps,
                tc.tile_pool(name="otps", bufs=2, space="PSUM") as otps,
                tc.tile_pool(name="epool", bufs=4) as epool,
                tc.tile_pool(name="attnp", bufs=6) as attnp,
                tc.tile_pool(name="etp", bufs=6) as etp,
                tc.tile_pool(name="onrmp", bufs=2) as onrmp,
            ):
                # attention-weights path: scores[t,s] -> exp -> normalize -> DMA
                a4s = {}
                for tn in range(NT):
                    for h in range(NH):
                        hs = slice(h * D, (h + 1) * D)
                        sc_ps = mmps.tile([P, T], f32, tag="sc", name=f"sc{h}_{tn}")
                        sq, sk = (qTr, kTr) if SCORES_F32R else (qT, kT)
                        for sch in range(2):
                            nc.tensor.matmul(
                                sc_ps[:, sch * SC : (sch + 1) * SC],
                                lhsT=sq[hs, tn * P : (tn + 1) * P],
                                rhs=sk[hs, sch * SC : (sch + 1) * SC],
                                start=True,
                                stop=True,
                                tile_position=(h * D, 0),
                            )
                        E = epool.tile([P, T], f32, tag="E", name=f"E{h}_{tn}")
                        idx = h * NT + tn
                        nc.scalar.activation(
                            E, sc_ps, EXP, accum_out=denom[:, idx : idx + 1]
                        )
                        nc.vector.reciprocal(
                            recip[:, idx : idx + 1], denom[:, idx : idx + 1]
                        )
                        if tn % 4 == 0:
                            a4s[h] = attnp.tile(
                                [P, 4, T], f32, tag="a4", name=f"a4_{h}_{tn // 4}"
                            )
                        nc.vector.tensor_scalar_mul(
                            a4s[h][:, tn % 4, :], E, recip[:, idx : idx + 1]
                        )
                        if tn % 4 == 3:
                            q4 = tn // 4
                            nc.sync.dma_start(
                                attn_d[h, q4 * 4 * P : (q4 + 1) * 4 * P, :].rearrange(
                                    "(n p) s -> p n s", p=P
                                ),
                                a4s[h],
                            )
                # o path: scoresT (fp32r) -> exp(fp32r) -> oT = v.T @ exp(scoresT)
                ET = etp.tile([P, NT, T], f32r, bufs=1)  # exp(scores.T) [s_p, sn, t]
                for h in range(NH):
                    hs = slice(h * D, (h + 1) * D)
                    for sn in range(NT):
                        scT_ps = mmps.tile([P, T], f32, tag="sc", name=f"scT{h}_{sn}")
                        for tch in range(2):
                            nc.tensor.matmul(
                                scT_ps[:, tch * SC : (tch + 1) * SC],
                                lhsT=kTr[hs, sn * P : (sn + 1) * P],
                                rhs=qTr[hs, tch * SC : (tch + 1) * SC],
                                start=True,
                                stop=True,
                                tile_position=(h * D, 0),
                            )
                        nc.scalar.activation(ET[:, sn, :], scT_ps, EXP)
                    for tch in range(2):
                        oT_ps = otps.tile([D, SC], f32, tag="ot", name=f"ot{h}_{tch}")
                        for sn in range(NT):
                            nc.tensor.matmul(
                                oT_ps,
                                lhsT=v3[:, sn, hs],
                                rhs=ET[:, sn, tch * SC : (tch + 1) * SC],
                                start=(sn == 0),
                                stop=(sn == NT - 1),
                            )
                        nc.vector.tensor_copy(
                            oT_sb[hs, tch * SC : (tch + 1) * SC], oT_ps
                        )

                # ---- out = gelu(o * recip) back in natural layout
                for tn in range(NT):
                    on_ps = mmps.tile([P, P], f32, tag="sc", name=f"on{tn}")
                    nc.tensor.transpose(on_ps, oT_sb[:, tn * P : (tn + 1) * P], ident)
                    onrm = onrmp.tile([P, NH, D], f32, tag="onrm")
                    rec4 = recip[:, tn : NH * NT : NT]  # [P, NH] head-major
                    nc.vector.tensor_tensor(
                        onrm,
                        on_ps.rearrange("p (h d) -> p h d", h=NH),
                        rec4[:, :, None].to_broadcast([P, NH, D]),
                        mybir.AluOpType.mult,
                    )
                    nc.scalar.activation(
                        ofin[:, tn, :], onrm.rearrange("p h d -> p (h d)"), GELU
                    )
                nc.sync.dma_start(out_d.rearrange("(n p) c -> p n c", p=P), ofin)

    nc.compile()
    return nc


def _prep_inputs(x, adj, W_qkv, eps):
    """Host-side shard/layout prep: one input map per core."""
    eps1 = np.full((P, 1), 1.0 + float(np.asarray(eps).reshape(-1)[0]), np.float32)
    wq = np.ascontiguousarray(W_qkv[:, 0:D]) * np.float32(SCALE)
    wk = np.ascontiguousarray(W_qkv[:, D : 2 * D])
    wv = np.ascontiguousarray(W_qkv[:, 2 * D : 3 * D])
    w4 = np.zeros((P, 2 * D), np.float32)
    wblk = np.zeros((P, P), np.float32)
    for h in range(NH):
        w4[h * D : (h + 1) * D, 0:D] = wq
        w4[h * D : (h + 1) * D, D : 2 * D] = wk
        wblk[h * D : (h + 1) * D, h * D : (h + 1) * D] = wv

    in_maps = []
    for core in range(8):
        b, hg = core // 2, core % 2
        xs = np.ascontiguousarray(x[b, :, hg * P : (hg + 1) * P])
        in_maps.append(
            {
                "adjT": np.ascontiguousarray(adj[b].T),
                "xs": xs,
                "xsT": np.ascontiguousarray(xs.T),
                "w4": w4,
                "wblk": wblk,
                "eps1": eps1,
            }
        )
    return in_maps


def run(x, adj, W_qkv, eps, trace=False):
    """Run on 8 NeuronCores; returns (out, attn_weight, BassKernelResults)."""
    from concourse import bass_utils

    if "nc" not in _CACHE:
        _CACHE["nc"] = _build()
    nc = _CACHE["nc"]

    in_maps = _prep_inputs(
        np.asarray(x, np.float32), np.asarray(adj, np.float32),
        np.asarray(W_qkv, np.float32), np.asarray(eps, np.float32),
    )
    res = bass_utils.run_bass_kernel_spmd(
        nc, in_maps, core_ids=list(range(8)), trace=trace
    )

    out = np.empty((B, T, DIM), np.float32)
    attn = np.empty((B, HEADS, T, T), np.float32)
    for core in range(8):
        b, hg = core // 2, core % 2
        r = res.results[core]
        attn[b, hg * NH : (hg + 1) * NH] = r["attn_o"]
        out[b, :, hg * P : (hg + 1) * P] = r["out_o"]
    return out, attn, res


def kernel(x, adj, rep_adj_dis, W_qkv, eps):
    out, attn, _ = run(x, adj, W_qkv, eps, trace=False)
    return out, attn
